# revision 21
# baseline (speedup 1.0000x reference)
"""Trainium2 Bass kernel for nn_GroupEncoder (bf16, 6-queue gather streaming).

Computes, for full inputs
    x:  (32, 128, 128, 128) f32
    r:  (32, 128, 128, 32)  f32
    w1: (128, 32, 8, 16)    f32
    w2: (32, 16, 8, 16)     f32
the reference:
    y = einsum('nijx,nijr->nrx', x, r)
    u = relu(einsum('nrx,xrvh->nrvh', y, w1) / (128*128))
    out = einsum('ruvh,nrvh->nruv', w2, u)        # (32, 32, 16, 8)

Sharding: data-parallel over n across 8 NeuronCores (4 samples/core),
w1/w2 replicated.  All tensors are cast to bf16 host-side (harness gate
is 2e-2 relative; bf16 lands ~4.5e-3), halving HBM traffic to ~22 MB/core.

The kernel is DMA-queue-bound; traffic is spread over six DMA queues:
4 SWDGE queues driven by gpsimd dma_gather (identity gather == strided
load; the gather path aggregates rows into ~128KB descriptors and runs
~120-240 GB/s/queue) plus the ACT HWDGE ring (~250-290 GB/s while the
gathers are still blocked).  dma_gather needs the mlp ucode library,
whose async reload (~16us after engine boot) keeps the gathers dark
until ~24us; the ACT ring covers that window.  Gather indices are built
on-chip (gpsimd iota + DVE fixup, no DMA).  Everything is SBUF-resident
(~193 KB/partition): all DMAs issue up front, the PE chases completions
with the per-sample i,j contraction (128 accumulating matmuls into
PSUM per sample), then a small w1/relu/w2 head at the tail.
"""

import numpy as np
import ml_dtypes

# Problem constants (hardcoded; kernel.py must be self-contained).
N, I, J = 32, 128, 128
XD, RD, UD, VD, HD = 128, 32, 16, 8, 16
NCORES = 8
NLOC = N // NCORES  # 4 samples per core
NORM = float(I * J)

# x j-chunking per sample: sample 0 in 4 quarter chunks (earliest PE start),
# samples 1-3 in halves.  (chunk_count, jc) per sample.
XCHUNK = [(4, 32), (2, 64), (2, 64), (2, 64)]

_cache = {}


def _build_nc():
    import concourse.mybir as mybir
    import concourse.tile as tile
    from concourse import bacc
    from concourse.library_config import mlp

    f32 = mybir.dt.float32
    bf16 = mybir.dt.bfloat16
    i16 = mybir.dt.int16
    Relu = mybir.ActivationFunctionType.Relu
    Alu = mybir.AluOpType

    nc = bacc.Bacc(
        "TRN2",
        target_bir_lowering=False,
        debug=False,
        num_devices=NCORES,
        num_swdge_queues=4,
    )
    x_d = nc.dram_tensor("x", [NLOC, I, J * XD], bf16, kind="ExternalInput").ap()
    r_d = nc.dram_tensor("r", [NLOC, I, J * RD], bf16, kind="ExternalInput").ap()
    w_d = nc.dram_tensor("wcat", [XD, 2 * RD * VD * HD], bf16, kind="ExternalInput").ap()
    out_d = nc.dram_tensor(
        "out", [UD * VD, RD * NLOC], f32, kind="ExternalOutput"
    ).ap()
    WOFF = RD * VD * HD  # w2bd column offset inside wcat

    with tile.TileContext(nc) as tc:
        with (
            tc.tile_pool(name="bp", bufs=1) as bp,
            tc.tile_pool(name="pp", bufs=1, space="PSUM") as pp,
        ):
            # ---- on-chip identity gather indices (wrapped in 16 partitions,
            # replicated for the 8 gpsimd cores): gidx[p, s] = (p % 16) + 16*s
            gidx = bp.tile([128, I // 16], i16, name="gidx")
            ip_t = bp.tile([128, I // 16], i16, name="ip_t")
            is_t = bp.tile([128, I // 16], i16, name="is_t")
            nc.gpsimd.iota(ip_t[:, :], [[0, I // 16]], channel_multiplier=1)
            nc.gpsimd.iota(is_t[:, :], [[16, I // 16]], channel_multiplier=0)
            nc.vector.tensor_scalar(ip_t[:, :], ip_t[:, :], 15, None, Alu.bitwise_and)
            nc.vector.tensor_tensor(gidx[:, :], ip_t[:, :], is_t[:, :], Alu.add)
            nc.gpsimd.load_library(mlp)

            wcat_sb = bp.tile([XD, 1, 2 * RD * VD * HD], bf16, name="wcat_sb")
            xt = [
                [
                    bp.tile([I, 1, jc * XD], bf16, name=f"xt_{n}_{c}")
                    for c in range(nch)
                ]
                for n, (nch, jc) in enumerate(XCHUNK)
            ]
            rt = [bp.tile([I, 1, J * RD], bf16, name=f"rt_{n}") for n in range(NLOC)]
            yT_sb = bp.tile([XD, RD, NLOC], bf16, name="yT_sb")
            u1_sb = bp.tile([VD * HD, RD * NLOC], bf16, name="u1_sb")
            out_sb = bp.tile([UD * VD, RD * NLOC], f32, name="out_sb")

            yp = [pp.tile([XD, RD], f32, name=f"yp_{n}") for n in range(NLOC)]
            u1ps = pp.tile([VD * HD, RD * NLOC], f32, name="u1ps")
            u2ps = pp.tile([UD * VD, RD * NLOC], f32, name="u2ps")

            def gx(q, n, c):  # gather one x chunk
                jc = XCHUNK[n][1]
                nc.gpsimd.dma_gather(
                    xt[n][c][:, :, :],
                    x_d[n, :, c * jc * XD : (c + 1) * jc * XD],
                    gidx[:, :],
                    I,
                    I,
                    jc * XD,
                    elem_step=J * XD,
                    queue_num=q,
                )

            def gr(q, n):  # gather one r sample
                nc.gpsimd.dma_gather(
                    rt[n][:, :, :],
                    r_d[n, :, :],
                    gidx[:, :],
                    I,
                    I,
                    J * RD,
                    queue_num=q,
                )

            def hx(eng, n, c):  # HWDGE load of one x chunk
                jc = XCHUNK[n][1]
                eng.dma_start(
                    xt[n][c][:, 0, :], x_d[n, :, c * jc * XD : (c + 1) * jc * XD]
                )

            # ---- queue schedule: issue everything up front ----
            # All SWDGE gathers are emitted BEFORE any HWDGE dma_start; each
            # ring's FIFO order matches the PE need order.  Gather queues get
            # exactly 4MB each; the ACT ring (which moves ~4.5MB for free
            # during the ~24us gather-dark window, then keeps streaming)
            # carries 6.3MB of late-needed chunks so no queue is a long pole.
            # q0: rt0 x01a rt2 rt3   q1: x00a x01b x20
            # q2: x00b rt1 wcat      q3: x10 x11
            # scalar: x21 x30 x31
            # Bytes per ring are inversely matched to observed ring speed:
            # q0 (~85 GB/s): rt0 x01a x01b   q1 (~115): x00a x20
            # q2 (~110): x00b rt1 wcat       q3 (~190): x10 x11 rt3
            # scalar: rt2 x21 x30 x31 (first ~4.5MB lands during the
            # gather-dark window for free).
            gr(0, 0)  # rt0 first: PE needs it immediately
            gx(1, 0, 0)  # x00a
            gx(2, 0, 1)  # x00b
            gx(3, 1, 0)  # x10
            gx(0, 0, 2)  # x01a
            gr(2, 1)  # rt1
            gx(3, 1, 1)  # x11
            gx(0, 0, 3)  # x01b
            gx(1, 2, 0)  # x20
            nc.gpsimd.dma_gather(  # wcat (w1 + w2bd)
                wcat_sb[:, :, :],
                w_d[:, :],
                gidx[:, :],
                I,
                I,
                2 * RD * VD * HD,
                queue_num=2,
            )
            gr(3, 3)  # rt3
            # ACT HWDGE ring: one mid r + the late-needed x chunks, in PE
            # need order.  The SP ring is cadence-limited (~28 GB/s bulk) so
            # it only carries the tiny `out` at the end.
            nc.scalar.dma_start(rt[2][:, 0, :], r_d[2, :, :])
            hx(nc.scalar, 2, 1)  # x(2,1)
            hx(nc.scalar, 3, 0)  # x(3,0)
            hx(nc.scalar, 3, 1)  # x(3,1)

            # ---- stage 1: y^T[x, r] = sum_ij x*r per sample ----
            for n in range(NLOC):
                nch, jc = XCHUNK[n]
                for c in range(nch):
                    for j in range(jc):
                        jj = c * jc + j
                        nc.tensor.matmul(
                            yp[n][:, :],
                            xt[n][c][:, 0, j * XD : (j + 1) * XD],
                            rt[n][:, 0, jj * RD : (jj + 1) * RD],
                            start=(jj == 0),
                            stop=(jj == J - 1),
                        )
                nc.scalar.copy(yT_sb[:, :, n], yp[n][:, :])

            # ---- stages 2+3 per sample (u1 = relu(w1_r^T y_r), out =
            # w2bd_r^T u1_r), emitted per sample in stage-1 completion order
            # so only the last sample's tiny head sits on the critical tail.
            for n in range(NLOC):
                for rr in range(RD):
                    nc.tensor.matmul(
                        u1ps[:, rr * NLOC + n : rr * NLOC + n + 1],
                        wcat_sb[:, 0, rr * VD * HD : (rr + 1) * VD * HD],
                        yT_sb[:, rr, n : n + 1],
                        start=True,
                        stop=True,
                    )
                nc.scalar.activation(u1_sb[:, n :: NLOC], u1ps[:, n :: NLOC], Relu)
                for rr in range(RD):
                    nc.tensor.matmul(
                        u2ps[:, rr * NLOC + n : rr * NLOC + n + 1],
                        wcat_sb[
                            :, 0, WOFF + rr * UD * VD : WOFF + (rr + 1) * UD * VD
                        ],
                        u1_sb[:, rr * NLOC + n : rr * NLOC + n + 1],
                        start=True,
                        stop=True,
                    )
                nc.scalar.copy(out_sb[:, n :: NLOC], u2ps[:, n :: NLOC])
            nc.sync.dma_start(out_d[:, :], out_sb[:, :])

    nc.compile()
    return nc


def _prep_in_maps(x, r, w1, w2):
    bf16 = ml_dtypes.bfloat16
    x = np.asarray(x, dtype=np.float32)
    r = np.asarray(r, dtype=np.float32)
    w1 = np.asarray(w1, dtype=np.float32)
    w2 = np.asarray(w2, dtype=np.float32)

    # Fold the 1/(i*j) normalization into w1.
    w1p = np.ascontiguousarray((w1 / NORM).reshape(XD, RD * VD * HD))
    # Block-diagonal expansion of w2 over v:
    # w2bd[(v h), r, (u v')] = w2[r, u, v, h] if v == v' else 0
    w2bd = np.zeros((RD, VD, HD, UD, VD), np.float32)
    for v in range(VD):
        w2bd[:, v, :, :, v] = np.transpose(w2[:, :, v, :], (0, 2, 1))
    w2bd = (
        w2bd.reshape(RD, VD * HD, UD * VD)
        .transpose(1, 0, 2)
        .reshape(VD * HD, RD * UD * VD)
    )
    wcat = np.ascontiguousarray(np.concatenate([w1p, w2bd], axis=1)).astype(bf16)

    x16 = x.astype(bf16).reshape(NCORES, NLOC, I, J * XD)
    r16 = r.astype(bf16).reshape(NCORES, NLOC, I, J * RD)

    in_maps = []
    for c in range(NCORES):
        in_maps.append(
            {
                "x": np.ascontiguousarray(x16[c]),
                "r": np.ascontiguousarray(r16[c]),
                "wcat": wcat,
            }
        )
    return in_maps


def _assemble(results):
    outs = []
    for c in range(NCORES):
        o = np.asarray(results[c]["out"], dtype=np.float32)  # [uv, (r n)]
        outs.append(o.reshape(UD, VD, RD, NLOC).transpose(3, 2, 0, 1))
    return np.ascontiguousarray(np.concatenate(outs, axis=0))


def run(x, r, w1, w2, **spmd_kwargs):
    """Build (cached), run on 8 cores, return (output, BassKernelResults)."""
    from concourse.bass_utils import run_bass_kernel_spmd

    if "nc" not in _cache:
        _cache["nc"] = _build_nc()
    nc = _cache["nc"]
    in_maps = _prep_in_maps(x, r, w1, w2)
    res = run_bass_kernel_spmd(
        nc, in_maps, core_ids=list(range(NCORES)), **spmd_kwargs
    )
    return _assemble(res.results), res


def kernel(x, r, w1, w2):
    out, _ = run(x, r, w1, w2)
    return out


# revision 24
# speedup vs baseline: 1.0948x; 1.0948x over previous
"""Trainium2 Bass kernel for nn_GroupEncoder (bf16, 6-queue gather streaming).

Computes, for full inputs
    x:  (32, 128, 128, 128) f32
    r:  (32, 128, 128, 32)  f32
    w1: (128, 32, 8, 16)    f32
    w2: (32, 16, 8, 16)     f32
the reference:
    y = einsum('nijx,nijr->nrx', x, r)
    u = relu(einsum('nrx,xrvh->nrvh', y, w1) / (128*128))
    out = einsum('ruvh,nrvh->nruv', w2, u)        # (32, 32, 16, 8)

Sharding: data-parallel over n across 8 NeuronCores (4 samples/core),
w1/w2 replicated.  All tensors are cast to bf16 host-side (harness gate
is 2e-2 relative; bf16 lands ~4.5e-3), halving HBM traffic to ~22 MB/core.

The kernel is DMA-queue-bound; traffic is spread over six DMA queues:
4 SWDGE queues driven by gpsimd dma_gather (identity gather == strided
load; the gather path aggregates rows into ~128KB descriptors and runs
~120-240 GB/s/queue) plus the ACT HWDGE ring (~250-290 GB/s while the
gathers are still blocked).  dma_gather needs the mlp ucode library,
whose async reload (~16us after engine boot) keeps the gathers dark
until ~24us; the ACT ring covers that window.  Gather indices are built
on-chip (gpsimd iota + DVE fixup, no DMA).  Everything is SBUF-resident
(~193 KB/partition): all DMAs issue up front, the PE chases completions
with the per-sample i,j contraction (128 accumulating matmuls into
PSUM per sample), then a small w1/relu/w2 head at the tail.
"""

import numpy as np
import ml_dtypes

# Problem constants (hardcoded; kernel.py must be self-contained).
N, I, J = 32, 128, 128
XD, RD, UD, VD, HD = 128, 32, 16, 8, 16
NCORES = 8
NLOC = N // NCORES  # 4 samples per core
NORM = float(I * J)

# x j-chunking per sample: sample 0 in 4 quarter chunks (earliest PE start),
# samples 1-3 in halves.  (chunk_count, jc) per sample.
XCHUNK = [(4, 32), (2, 64), (2, 64), (2, 64)]

_cache = {}


def _build_nc():
    import concourse.mybir as mybir
    import concourse.tile as tile
    from concourse import bacc
    from concourse.library_config import mlp

    f32 = mybir.dt.float32
    bf16 = mybir.dt.bfloat16
    i16 = mybir.dt.int16
    Relu = mybir.ActivationFunctionType.Relu
    Alu = mybir.AluOpType

    nc = bacc.Bacc(
        "TRN2",
        target_bir_lowering=False,
        debug=False,
        num_devices=NCORES,
        num_swdge_queues=4,
    )
    x_d = nc.dram_tensor("x", [NLOC, I, J * XD], bf16, kind="ExternalInput").ap()
    r_d = nc.dram_tensor("r", [NLOC, I, J * RD], bf16, kind="ExternalInput").ap()
    w_d = nc.dram_tensor("wcat", [XD, 2 * RD * VD * HD], bf16, kind="ExternalInput").ap()
    out_d = nc.dram_tensor(
        "out", [UD * VD, RD * NLOC], f32, kind="ExternalOutput"
    ).ap()
    WOFF = RD * VD * HD  # w2bd column offset inside wcat

    with tile.TileContext(nc) as tc:
        with (
            tc.tile_pool(name="bp", bufs=1) as bp,
            tc.tile_pool(name="pp", bufs=1, space="PSUM") as pp,
        ):
            # ---- on-chip identity gather indices (wrapped in 16 partitions,
            # replicated for the 8 gpsimd cores): gidx[p, s] = (p % 16) + 16*s
            gidx = bp.tile([128, I // 16], i16, name="gidx")
            ip_t = bp.tile([128, I // 16], i16, name="ip_t")
            is_t = bp.tile([128, I // 16], i16, name="is_t")
            nc.gpsimd.iota(ip_t[:, :], [[0, I // 16]], channel_multiplier=1)
            nc.gpsimd.iota(is_t[:, :], [[16, I // 16]], channel_multiplier=0)
            nc.vector.tensor_scalar(ip_t[:, :], ip_t[:, :], 15, None, Alu.bitwise_and)
            nc.vector.tensor_tensor(gidx[:, :], ip_t[:, :], is_t[:, :], Alu.add)

            wcat_sb = bp.tile([XD, 1, 2 * RD * VD * HD], bf16, name="wcat_sb")
            xt = [
                [
                    bp.tile([I, 1, jc * XD], bf16, name=f"xt_{n}_{c}")
                    for c in range(nch)
                ]
                for n, (nch, jc) in enumerate(XCHUNK)
            ]
            rt = [bp.tile([I, 1, J * RD], bf16, name=f"rt_{n}") for n in range(NLOC)]
            yT_sb = bp.tile([XD, RD, NLOC], bf16, name="yT_sb")
            u1_sb = bp.tile([VD * HD, RD * NLOC], bf16, name="u1_sb")
            out_sb = bp.tile([UD * VD, RD * NLOC], f32, name="out_sb")

            yp = [pp.tile([XD, RD], f32, name=f"yp_{n}") for n in range(NLOC)]
            u1ps = pp.tile([VD * HD, RD * NLOC], f32, name="u1ps")
            u2ps = pp.tile([UD * VD, RD * NLOC], f32, name="u2ps")

            def gx(q, n, c):  # gather one x chunk
                jc = XCHUNK[n][1]
                nc.gpsimd.dma_gather(
                    xt[n][c][:, :, :],
                    x_d[n, :, c * jc * XD : (c + 1) * jc * XD],
                    gidx[:, :],
                    I,
                    I,
                    jc * XD,
                    elem_step=J * XD,
                    queue_num=q,
                )

            def gr(q, n):  # gather one r sample
                nc.gpsimd.dma_gather(
                    rt[n][:, :, :],
                    r_d[n, :, :],
                    gidx[:, :],
                    I,
                    I,
                    J * RD,
                    queue_num=q,
                )

            def hx(eng, n, c):  # HWDGE load of one x chunk
                jc = XCHUNK[n][1]
                eng.dma_start(
                    xt[n][c][:, 0, :], x_d[n, :, c * jc * XD : (c + 1) * jc * XD]
                )

            # ---- queue schedule: issue everything up front ----
            # All SWDGE gathers are emitted BEFORE any HWDGE dma_start; each
            # ring's FIFO order matches the PE need order.  Gather queues get
            # exactly 4MB each; the ACT ring (which moves ~4.5MB for free
            # during the ~24us gather-dark window, then keeps streaming)
            # carries 6.3MB of late-needed chunks so no queue is a long pole.
            # q0: rt0 x01a rt2 rt3   q1: x00a x01b x20
            # q2: x00b rt1 wcat      q3: x10 x11
            # scalar: x21 x30 x31
            # Bytes per ring are inversely matched to observed ring speed,
            # and the two tail-gating tensors are taken off the gather
            # queues: wcat (stage-2 gate) rides the ACT ring's free
            # gather-dark window, and rt0 rides SWDGE q0 via plain dma_start
            # issued BEFORE load_library (base-library op, desc-gen ~7.5us).
            # q0 (~85 GB/s): [rt0-ds] x01a x01b   q1 (~115): x00a x20
            # q2 (~110): x00b rt1                 q3 (~190): x10 x11 rt3
            # scalar: wcat rt2 x21 x30 x31 (ends ~52us; x31 is the last
            # PE-needed chunk, so the tail after it is short).
            nc.gpsimd.dma_start(rt[0][:, 0, :], r_d[0, :, :])  # q0, pre-mlp
            nc.gpsimd.load_library(mlp)
            gx(1, 0, 0)  # x00a
            gx(2, 0, 1)  # x00b
            gx(3, 1, 0)  # x10
            gx(0, 0, 2)  # x01a
            gr(2, 1)  # rt1
            gx(3, 1, 1)  # x11
            gx(0, 0, 3)  # x01b
            gx(1, 2, 0)  # x20
            gr(3, 3)  # rt3
            # ACT HWDGE ring, in PE need order.  The SP ring is
            # cadence-limited (~28 GB/s bulk): only the tiny `out`.
            nc.scalar.dma_start(wcat_sb[:, 0, :], w_d[:, :])
            nc.scalar.dma_start(rt[2][:, 0, :], r_d[2, :, :])
            hx(nc.scalar, 2, 1)  # x(2,1)
            hx(nc.scalar, 3, 0)  # x(3,0)
            hx(nc.scalar, 3, 1)  # x(3,1)

            # ---- stage 1: y^T[x, r] = sum_ij x*r per sample ----
            for n in range(NLOC):
                nch, jc = XCHUNK[n]
                for c in range(nch):
                    for j in range(jc):
                        jj = c * jc + j
                        nc.tensor.matmul(
                            yp[n][:, :],
                            xt[n][c][:, 0, j * XD : (j + 1) * XD],
                            rt[n][:, 0, jj * RD : (jj + 1) * RD],
                            start=(jj == 0),
                            stop=(jj == J - 1),
                        )
                nc.scalar.copy(yT_sb[:, :, n], yp[n][:, :])

            # ---- stage 2: u1[vh, (r n)] = relu(w1_r^T y_r / norm) ----
            # (Batched across samples: splitting per sample costs 4x the
            # cross-engine relu round-trips and measures ~6us slower.)
            for rr in range(RD):
                nc.tensor.matmul(
                    u1ps[:, rr * NLOC : (rr + 1) * NLOC],
                    wcat_sb[:, 0, rr * VD * HD : (rr + 1) * VD * HD],
                    yT_sb[:, rr, :],
                    start=True,
                    stop=True,
                )
            nc.scalar.activation(u1_sb[:, :], u1ps[:, :], Relu)
            # ---- stage 3: out[uv, (r n)] = w2bd_r^T u1_r ----
            for rr in range(RD):
                nc.tensor.matmul(
                    u2ps[:, rr * NLOC : (rr + 1) * NLOC],
                    wcat_sb[:, 0, WOFF + rr * UD * VD : WOFF + (rr + 1) * UD * VD],
                    u1_sb[:, rr * NLOC : (rr + 1) * NLOC],
                    start=True,
                    stop=True,
                )
            nc.scalar.copy(out_sb[:, :], u2ps[:, :])
            nc.sync.dma_start(out_d[:, :], out_sb[:, :])

    nc.compile()
    return nc


def _prep_in_maps(x, r, w1, w2):
    bf16 = ml_dtypes.bfloat16
    x = np.asarray(x, dtype=np.float32)
    r = np.asarray(r, dtype=np.float32)
    w1 = np.asarray(w1, dtype=np.float32)
    w2 = np.asarray(w2, dtype=np.float32)

    # Fold the 1/(i*j) normalization into w1.
    w1p = np.ascontiguousarray((w1 / NORM).reshape(XD, RD * VD * HD))
    # Block-diagonal expansion of w2 over v:
    # w2bd[(v h), r, (u v')] = w2[r, u, v, h] if v == v' else 0
    w2bd = np.zeros((RD, VD, HD, UD, VD), np.float32)
    for v in range(VD):
        w2bd[:, v, :, :, v] = np.transpose(w2[:, :, v, :], (0, 2, 1))
    w2bd = (
        w2bd.reshape(RD, VD * HD, UD * VD)
        .transpose(1, 0, 2)
        .reshape(VD * HD, RD * UD * VD)
    )
    wcat = np.ascontiguousarray(np.concatenate([w1p, w2bd], axis=1)).astype(bf16)

    x16 = x.astype(bf16).reshape(NCORES, NLOC, I, J * XD)
    r16 = r.astype(bf16).reshape(NCORES, NLOC, I, J * RD)

    in_maps = []
    for c in range(NCORES):
        in_maps.append(
            {
                "x": np.ascontiguousarray(x16[c]),
                "r": np.ascontiguousarray(r16[c]),
                "wcat": wcat,
            }
        )
    return in_maps


def _assemble(results):
    outs = []
    for c in range(NCORES):
        o = np.asarray(results[c]["out"], dtype=np.float32)  # [uv, (r n)]
        outs.append(o.reshape(UD, VD, RD, NLOC).transpose(3, 2, 0, 1))
    return np.ascontiguousarray(np.concatenate(outs, axis=0))


def run(x, r, w1, w2, **spmd_kwargs):
    """Build (cached), run on 8 cores, return (output, BassKernelResults)."""
    from concourse.bass_utils import run_bass_kernel_spmd

    if "nc" not in _cache:
        _cache["nc"] = _build_nc()
    nc = _cache["nc"]
    in_maps = _prep_in_maps(x, r, w1, w2)
    res = run_bass_kernel_spmd(
        nc, in_maps, core_ids=list(range(NCORES)), **spmd_kwargs
    )
    return _assemble(res.results), res


def kernel(x, r, w1, w2):
    out, _ = run(x, r, w1, w2)
    return out
